# revision 11
# baseline (speedup 1.0000x reference)
"""Trainium2 Bass kernel for nn_Discriminator_15668040696127.

Computes:
    q, a, d = samples[:, 0], samples[:, 1], samples[:, 2]        # [B, D]
    cos1 = <q,d> / max(||q||*||d||, 1e-6)                         # [B]
    cos2 = <a,d> / max(||a||*||d||, 1e-6)                         # [B]
    score = cos1 @ D_v1 + cos2 @ D_v2                             # scalar
    out = BCE_with_logits(score, labels[0])                       # scalar

Sharding: data-parallel over B across 8 NeuronCores (1024 samples
each).  Each core streams its 48 MiB sample shard and reduces it to a
single partial-score float; the host sums the 8 partials and applies
the scalar BCE.  No device collective: the SPMD dispatch (one PJRT
shard_map over 8 axon devices) can start cores 100+ us apart, and any
cross-core dependency puts that full skew into every earlier core's
measured exec time (observed 200-315 us run-to-run with an on-device
all-reduce of the same math).

The stream runs anywhere from ~128 us (paired-NC HBM stack idle,
~394 GB/s) to ~148 us, so every engine's per-tile work is kept below
the fast-case per-tile stream time (~15.0 us per 6 MB tile):
  - ACT: qq, aa squares + the low half of dd   (~12.4 us)
  - DVE: qd, ad dots + the high half of dd     (~12.2 us)
(gpsimd only does the tiny dvb loads: TENSOR_SCALAR_PTR is not a
valid Pool opcode, so it cannot take compute passes.)

Tile component order is q,d,a so per-tile DVE work starts as early as
possible and the queue is drained when the tail begins.  d6,q6,d7,q7
are hoisted to the head of the stream (their dd/qq/qd run during
tiles 0..1) and a6/a7 arrive last as eight 1 MB chunks, ad->DVE,
aa->ACT, so only ~2 us of chunk work trails the final byte.  The cos
epilogue (sqrt/recip/weighting) is one batched [P,16] pass; its
activation-table switch loads while the last chunks drain.
"""

import os
import sys

import numpy as np

for _p in ("/opt/trn_rl_repo", "/root/.axon_site/_ro/trn_rl_repo"):
    if os.path.isdir(_p) and _p not in sys.path:
        sys.path.append(_p)

import concourse.bass as bass
import concourse.bacc as bacc
import concourse.mybir as mybir
import concourse.tile as tile
from concourse import bass_utils

N_CORES = 8
B, D = 8192, 4096
BS = B // N_CORES          # 1024 samples per core
P = 128                    # SBUF partitions
T = BS // P                # 8 tiles of 128 samples per core
EPS = 1e-6
NCH = 4                    # a-chunks for each of the last two tiles
CH = D // NCH
H = D // 2                 # dd half-split point

f32 = mybir.dt.float32
Alu = mybir.AluOpType
Act = mybir.ActivationFunctionType

_CACHE = {}


def _build_program():
    nc = bacc.Bacc(
        "TRN2",
        target_bir_lowering=False,
        debug=False,
        num_devices=N_CORES,
    )

    samples = nc.dram_tensor("samples", [BS, 3, D], f32, kind="ExternalInput")
    dv1 = nc.dram_tensor("dv1", [BS], f32, kind="ExternalInput")
    dv2 = nc.dram_tensor("dv2", [BS], f32, kind="ExternalInput")
    out = nc.dram_tensor("out", [2 * T, 1], f32, kind="ExternalOutput")

    with tile.TileContext(nc) as tc:
        with (
            tc.tile_pool(name="dp", bufs=3) as d_pool,
            tc.tile_pool(name="qp", bufs=2) as q_pool,
            tc.tile_pool(name="ap", bufs=2) as a_pool,
            tc.tile_pool(name="junk", bufs=1) as junk_pool,
            tc.tile_pool(name="stats", bufs=1) as stats_pool,
            tc.tile_pool(name="psum", bufs=1, space="PSUM") as psum_pool,
        ):
            # Stats columns: col t = tile t's q-vs-d stat, col T+t = a-vs-d.
            dots = stats_pool.tile([P, 2 * T], f32, tag="dots")
            nprod = stats_pool.tile([P, 2 * T], f32, tag="nprod")
            contrib = stats_pool.tile([P, 2 * T], f32, tag="contrib")
            dvb = stats_pool.tile([P, 2 * T], f32, tag="dvb")

            def act_sq_accum(src, acc, sl=None):
                ja = junk_pool.tile([P, D], f32, tag="junk_act")
                o, i = (ja[:], src) if sl is None else (ja[:, sl], src)
                nc.scalar.activation(
                    out=o, in_=i, func=Act.Square, accum_out=acc[:]
                )

            def dve_dot_accum(src0, src1, acc, sl=None):
                jv = junk_pool.tile([P, D], f32, tag="junk_dve")
                o = jv[:] if sl is None else jv[:, sl]
                nc.vector.scalar_tensor_tensor(
                    out=o, in0=src0, scalar=1.0, in1=src1,
                    op0=Alu.mult, op1=Alu.mult, accum_out=acc[:],
                )

            # --- Head: d/q of the last two tiles stream first; their
            # dd/qq/qd work runs while tiles 0..1 stream.
            d6 = stats_pool.tile([P, D], f32, tag="d6")
            q6 = q_pool.tile([P, D], f32, tag="q")
            d7 = stats_pool.tile([P, D], f32, tag="d7")
            q7 = q_pool.tile([P, D], f32, tag="q")
            nc.sync.dma_start(d6[:], samples[bass.ts(T - 2, P), 2, :])
            nc.sync.dma_start(q6[:], samples[bass.ts(T - 2, P), 0, :])
            nc.sync.dma_start(d7[:], samples[bass.ts(T - 1, P), 2, :])
            nc.sync.dma_start(q7[:], samples[bass.ts(T - 1, P), 0, :])

            # Small loads ride the SWDGE path, off the HWDGE stream queue.
            nc.gpsimd.dma_start(dvb[:, 0:T], dv1[:].rearrange("(n p) -> p n", p=P))
            nc.gpsimd.dma_start(
                dvb[:, T : 2 * T], dv2[:].rearrange("(n p) -> p n", p=P)
            )
            ones = stats_pool.tile([P, 1], f32, tag="ones")
            nc.gpsimd.memset(ones[:], 1.0)

            # Head compute: dd6/dd7 whole on ACT (idle in the head),
            # qd6/qd7 on DVE.  dots columns carry the dvb weight already
            # (dot*w) so the tail epilogue skips a [P,16] mul.
            dd6 = stats_pool.tile([P, 1], f32, tag="dd6")
            dd7 = stats_pool.tile([P, 1], f32, tag="dd7")
            for t, qt, dt, ddt in (
                (T - 2, q6, d6, dd6),
                (T - 1, q7, d7, dd7),
            ):
                act_sq_accum(dt[:], ddt)
                qq = junk_pool.tile([P, 1], f32, tag=f"qq{t}")
                act_sq_accum(qt[:], qq)
                qd = junk_pool.tile([P, 1], f32, tag=f"qd{t}")
                dve_dot_accum(qt[:], dt[:], qd)
                nc.vector.tensor_mul(dots[:, t : t + 1], qd[:], dvb[:, t : t + 1])
                nc.vector.tensor_mul(nprod[:, t : t + 1], qq[:], ddt[:])

            # --- Tiles 0..5: q, d, a component DMAs (q first so DVE's qd
            # can start at d-arrival and is long done when a lands).
            for t in range(T - 2):
                q_t = q_pool.tile([P, D], f32, tag="q")
                d_t = d_pool.tile([P, D], f32, tag="d")
                a_t = a_pool.tile([P, D], f32, tag="a")
                nc.sync.dma_start(q_t[:], samples[bass.ts(t, P), 0, :])
                nc.sync.dma_start(d_t[:], samples[bass.ts(t, P), 2, :])
                nc.sync.dma_start(a_t[:], samples[bass.ts(t, P), 1, :])

                # dd split: low half ACT square, high half DVE stt.
                dd_a = junk_pool.tile([P, 1], f32, tag="dd_a")
                act_sq_accum(d_t[:, 0:H], dd_a, slice(0, H))
                dd_v = junk_pool.tile([P, 1], f32, tag="dd_v")
                dve_dot_accum(d_t[:, H:D], d_t[:, H:D], dd_v, slice(H, D))
                dd = junk_pool.tile([P, 1], f32, tag="dd")
                nc.vector.tensor_add(dd[:], dd_a[:], dd_v[:])

                qd = junk_pool.tile([P, 1], f32, tag="qd")
                dve_dot_accum(q_t[:], d_t[:], qd)
                nc.vector.tensor_mul(dots[:, t : t + 1], qd[:], dvb[:, t : t + 1])
                qq = junk_pool.tile([P, 1], f32, tag="qq")
                act_sq_accum(q_t[:], qq)
                nc.vector.tensor_mul(nprod[:, t : t + 1], qq[:], dd[:])

                ad = junk_pool.tile([P, 1], f32, tag="ad")
                dve_dot_accum(a_t[:], d_t[:], ad)
                nc.vector.tensor_mul(
                    dots[:, T + t : T + t + 1], ad[:], dvb[:, T + t : T + t + 1]
                )
                aa = junk_pool.tile([P, 1], f32, tag="aa")
                act_sq_accum(a_t[:], aa)
                nc.vector.tensor_mul(nprod[:, T + t : T + t + 1], aa[:], dd[:])

            # --- Tail: a6 (4 x 1 MB chunks) then a7 (3 x 1.25 MB + one
            # small 256-col chunk so almost no compute trails the final
            # byte).  ad chunks on DVE, aa chunks on ACT, except a7's
            # last aa chunk on DVE so ACT's sqrt-table load starts while
            # the last chunks drain.
            a6 = a_pool.tile([P, D], f32, tag="a")
            a7 = a_pool.tile([P, D], f32, tag="a")
            A7B = [0, 1280, 2560, 3840, D]   # a7 chunk boundaries
            for k in range(NCH):
                sl = slice(k * CH, (k + 1) * CH)
                nc.sync.dma_start(a6[:, sl], samples[bass.ts(T - 2, P), 1, sl])
            for k in range(NCH):
                sl = slice(A7B[k], A7B[k + 1])
                nc.sync.dma_start(a7[:, sl], samples[bass.ts(T - 1, P), 1, sl])

            def chunk_chain(name, emit_one, bounds=None):
                accs = []
                for k in range(NCH):
                    if bounds is None:
                        sl = slice(k * CH, (k + 1) * CH)
                    else:
                        sl = slice(bounds[k], bounds[k + 1])
                    acc = junk_pool.tile([P, 1], f32, tag=f"ch_{name}_{k}")
                    emit_one(k, sl, acc)
                    accs.append(acc)
                    if k > 0:
                        nc.vector.tensor_add(accs[k][:], accs[k][:], accs[k - 1][:])
                return accs[-1]

            ad6 = chunk_chain(
                "ad6", lambda k, sl, acc: dve_dot_accum(a6[:, sl], d6[:, sl], acc, sl)
            )
            aa6 = chunk_chain(
                "aa6", lambda k, sl, acc: act_sq_accum(a6[:, sl], acc, sl)
            )
            ad7 = chunk_chain(
                "ad7",
                lambda k, sl, acc: dve_dot_accum(a7[:, sl], d7[:, sl], acc, sl),
                bounds=A7B,
            )

            def aa7_emit(k, sl, acc):
                if k < NCH - 1:
                    act_sq_accum(a7[:, sl], acc, sl)
                else:
                    dve_dot_accum(a7[:, sl], a7[:, sl], acc, sl)

            aa7 = chunk_chain("aa7", aa7_emit, bounds=A7B)

            c6, c7 = T - 2, T - 1
            nc.vector.tensor_mul(
                dots[:, T + c6 : T + c6 + 1], ad6[:], dvb[:, T + c6 : T + c6 + 1]
            )
            nc.vector.tensor_mul(nprod[:, T + c6 : T + c6 + 1], aa6[:], dd6[:])
            nc.vector.tensor_mul(
                dots[:, T + c7 : T + c7 + 1], ad7[:], dvb[:, T + c7 : T + c7 + 1]
            )
            nc.vector.tensor_mul(nprod[:, T + c7 : T + c7 + 1], aa7[:], dd7[:])

            # --- Batched cos epilogue over all 16 columns (dots already
            # carry the dvb weights): w*cos = dots / max(sqrt(nprod), EPS).
            norm = stats_pool.tile([P, 2 * T], f32, tag="norm")
            nc.scalar.activation(norm[:], nprod[:], Act.Sqrt)
            nc.vector.tensor_scalar_max(norm[:], norm[:], EPS)
            nc.vector.reciprocal(norm[:], norm[:])
            nc.vector.tensor_mul(contrib[:], dots[:], norm[:])

            # Partition reduce: psum[c,0] = sum_p contrib[p,c]; the host
            # sums the 16 column partials per core.
            psum_t = psum_pool.tile([2 * T, 1], f32, tag="psum_s")
            nc.tensor.matmul(psum_t[:], contrib[:], ones[:], start=True, stop=True)
            partial = stats_pool.tile([2 * T, 1], f32, tag="partial")
            nc.vector.tensor_copy(partial[:], psum_t[:])
            nc.sync.dma_start(out[:], partial[:])

    nc.compile()
    return nc


def _get_program():
    if "nc" not in _CACHE:
        _CACHE["nc"] = _build_program()
    return _CACHE["nc"]


def kernel(samples, labels, D_v1, D_v2):
    samples = np.asarray(samples, dtype=np.float32)
    labels = np.asarray(labels, dtype=np.float32)
    D_v1 = np.asarray(D_v1, dtype=np.float32)
    D_v2 = np.asarray(D_v2, dtype=np.float32)
    assert samples.shape == (B, 3, D), samples.shape

    nc = _get_program()

    in_maps = []
    for c in range(N_CORES):
        sl = slice(c * BS, (c + 1) * BS)
        in_maps.append(
            {
                "samples": np.ascontiguousarray(samples[sl]),
                "dv1": np.ascontiguousarray(D_v1[sl]),
                "dv2": np.ascontiguousarray(D_v2[sl]),
            }
        )

    _tc = os.environ.get("KERNEL_TRACE_CORES")
    _kw = {"trace_cores": [int(x) for x in _tc.split(",")]} if _tc else {}
    try:
        res = bass_utils.run_bass_kernel_spmd(
            nc, in_maps, core_ids=list(range(N_CORES)), **_kw
        )
    except Exception:
        # A previously-wedged NeuronCore surfaces as an unrecoverable
        # exec error on the first attempt; the runtime resets it, so a
        # single retry recovers.
        res = bass_utils.run_bass_kernel_spmd(
            nc, in_maps, core_ids=list(range(N_CORES)), **_kw
        )
    _CACHE["last_results"] = res

    # Host-side unshard: sum the per-core column partials into the scalar
    # score, then the scalar BCE.
    score = float(
        sum(
            np.asarray(res.results[c]["out"], dtype=np.float64).sum()
            for c in range(N_CORES)
        )
    )
    y = float(labels.reshape(-1)[0])
    bce = max(score, 0.0) - score * y + np.log1p(np.exp(-abs(score)))
    return np.float32(bce).reshape(())


# revision 12
# speedup vs baseline: 1.0508x; 1.0508x over previous
"""Trainium2 Bass kernel for nn_Discriminator_15668040696127.

Computes:
    q, a, d = samples[:, 0], samples[:, 1], samples[:, 2]        # [B, D]
    cos1 = <q,d> / max(||q||*||d||, 1e-6)                         # [B]
    cos2 = <a,d> / max(||a||*||d||, 1e-6)                         # [B]
    score = cos1 @ D_v1 + cos2 @ D_v2                             # scalar
    out = BCE_with_logits(score, labels[0])                       # scalar

Sharding: data-parallel over B across 8 NeuronCores (1024 samples
each).  Each core streams its 48 MiB sample shard and reduces it to a
single partial-score float; the host sums the 8 partials and applies
the scalar BCE.  No device collective: the SPMD dispatch (one PJRT
shard_map over 8 axon devices) can start cores 100+ us apart, and any
cross-core dependency puts that full skew into every earlier core's
measured exec time (observed 200-315 us run-to-run with an on-device
all-reduce of the same math).

The stream runs anywhere from ~128 us (paired-NC HBM stack idle,
~394 GB/s) to ~148 us, so every engine's per-tile work is kept below
the fast-case per-tile stream time (~15.0 us per 6 MB tile):
  - ACT: qq, aa squares + the low half of dd   (~12.4 us)
  - DVE: qd, ad dots + the high half of dd     (~12.2 us)
(gpsimd only does the tiny dvb loads: TENSOR_SCALAR_PTR is not a
valid Pool opcode, so it cannot take compute passes.)

Tile component order is q,d,a so per-tile DVE work starts as early as
possible and the queue is drained when the tail begins.  d6,q6,d7,q7
are hoisted to the head of the stream (their dd/qq/qd run during
tiles 0..1) and a6/a7 arrive last in chunks (a6: 4 x 1 MB; a7:
3 x 1.25 MB + one 256-col runt), ad->DVE, aa->ACT except a7's last
aa chunk on DVE, so ACT's sqrt-table load and the final chunk passes
overlap and <1.5 us of compute trails the final byte.  dots columns
carry the D_v1/D_v2 weights as they are produced; the cos epilogue is
sqrt -> max -> recip -> mul on [P,16], a [16,1] PE partition-reduce,
and a 64 B output DMA (host sums 8x16 partials + scalar BCE).

Measured (core 0, ntff): 144.8-169.7 us over 4 runs; the spread is
HBM-stack co-tenancy (stream runs 356-397 GB/s run to run).  At equal
stream rate this kernel's fixed overhead is ~17 us (8.1 preamble +
~6.5 tail incl. out-DMA + ~2.5 teardown) vs ~73 us for the baseline.
"""

import os
import sys

import numpy as np

for _p in ("/opt/trn_rl_repo", "/root/.axon_site/_ro/trn_rl_repo"):
    if os.path.isdir(_p) and _p not in sys.path:
        sys.path.append(_p)

import concourse.bass as bass
import concourse.bacc as bacc
import concourse.mybir as mybir
import concourse.tile as tile
from concourse import bass_utils

N_CORES = 8
B, D = 8192, 4096
BS = B // N_CORES          # 1024 samples per core
P = 128                    # SBUF partitions
T = BS // P                # 8 tiles of 128 samples per core
EPS = 1e-6
NCH = 4                    # a-chunks for each of the last two tiles
CH = D // NCH
H = D // 2                 # dd half-split point

f32 = mybir.dt.float32
Alu = mybir.AluOpType
Act = mybir.ActivationFunctionType

_CACHE = {}


def _build_program():
    nc = bacc.Bacc(
        "TRN2",
        target_bir_lowering=False,
        debug=False,
        num_devices=N_CORES,
    )

    samples = nc.dram_tensor("samples", [BS, 3, D], f32, kind="ExternalInput")
    dv1 = nc.dram_tensor("dv1", [BS], f32, kind="ExternalInput")
    dv2 = nc.dram_tensor("dv2", [BS], f32, kind="ExternalInput")
    out = nc.dram_tensor("out", [2 * T, 1], f32, kind="ExternalOutput")

    with tile.TileContext(nc) as tc:
        with (
            tc.tile_pool(name="dp", bufs=3) as d_pool,
            tc.tile_pool(name="qp", bufs=2) as q_pool,
            tc.tile_pool(name="ap", bufs=2) as a_pool,
            tc.tile_pool(name="junk", bufs=1) as junk_pool,
            tc.tile_pool(name="stats", bufs=1) as stats_pool,
            tc.tile_pool(name="psum", bufs=1, space="PSUM") as psum_pool,
        ):
            # Stats columns: col t = tile t's q-vs-d stat, col T+t = a-vs-d.
            dots = stats_pool.tile([P, 2 * T], f32, tag="dots")
            nprod = stats_pool.tile([P, 2 * T], f32, tag="nprod")
            contrib = stats_pool.tile([P, 2 * T], f32, tag="contrib")
            dvb = stats_pool.tile([P, 2 * T], f32, tag="dvb")

            def act_sq_accum(src, acc, sl=None):
                ja = junk_pool.tile([P, D], f32, tag="junk_act")
                o, i = (ja[:], src) if sl is None else (ja[:, sl], src)
                nc.scalar.activation(
                    out=o, in_=i, func=Act.Square, accum_out=acc[:]
                )

            def dve_dot_accum(src0, src1, acc, sl=None):
                jv = junk_pool.tile([P, D], f32, tag="junk_dve")
                o = jv[:] if sl is None else jv[:, sl]
                nc.vector.scalar_tensor_tensor(
                    out=o, in0=src0, scalar=1.0, in1=src1,
                    op0=Alu.mult, op1=Alu.mult, accum_out=acc[:],
                )

            # --- Head: d/q of the last two tiles stream first; their
            # dd/qq/qd work runs while tiles 0..1 stream.
            d6 = stats_pool.tile([P, D], f32, tag="d6")
            q6 = q_pool.tile([P, D], f32, tag="q")
            d7 = stats_pool.tile([P, D], f32, tag="d7")
            q7 = q_pool.tile([P, D], f32, tag="q")
            nc.sync.dma_start(d6[:], samples[bass.ts(T - 2, P), 2, :])
            nc.sync.dma_start(q6[:], samples[bass.ts(T - 2, P), 0, :])
            nc.sync.dma_start(d7[:], samples[bass.ts(T - 1, P), 2, :])
            nc.sync.dma_start(q7[:], samples[bass.ts(T - 1, P), 0, :])

            # Small loads ride the SWDGE path, off the HWDGE stream queue.
            nc.gpsimd.dma_start(dvb[:, 0:T], dv1[:].rearrange("(n p) -> p n", p=P))
            nc.gpsimd.dma_start(
                dvb[:, T : 2 * T], dv2[:].rearrange("(n p) -> p n", p=P)
            )
            ones = stats_pool.tile([P, 1], f32, tag="ones")
            nc.gpsimd.memset(ones[:], 1.0)

            # Head compute: dd6/dd7 whole on ACT (idle in the head),
            # qd6/qd7 on DVE.  dots columns carry the dvb weight already
            # (dot*w) so the tail epilogue skips a [P,16] mul.
            dd6 = stats_pool.tile([P, 1], f32, tag="dd6")
            dd7 = stats_pool.tile([P, 1], f32, tag="dd7")
            for t, qt, dt, ddt in (
                (T - 2, q6, d6, dd6),
                (T - 1, q7, d7, dd7),
            ):
                act_sq_accum(dt[:], ddt)
                qq = junk_pool.tile([P, 1], f32, tag=f"qq{t}")
                act_sq_accum(qt[:], qq)
                qd = junk_pool.tile([P, 1], f32, tag=f"qd{t}")
                dve_dot_accum(qt[:], dt[:], qd)
                nc.vector.tensor_mul(dots[:, t : t + 1], qd[:], dvb[:, t : t + 1])
                nc.vector.tensor_mul(nprod[:, t : t + 1], qq[:], ddt[:])

            # --- Tiles 0..5: q, d, a component DMAs (q first so DVE's qd
            # can start at d-arrival and is long done when a lands).
            for t in range(T - 2):
                q_t = q_pool.tile([P, D], f32, tag="q")
                d_t = d_pool.tile([P, D], f32, tag="d")
                a_t = a_pool.tile([P, D], f32, tag="a")
                nc.sync.dma_start(q_t[:], samples[bass.ts(t, P), 0, :])
                nc.sync.dma_start(d_t[:], samples[bass.ts(t, P), 2, :])
                nc.sync.dma_start(a_t[:], samples[bass.ts(t, P), 1, :])

                # dd split: low half ACT square, high half DVE stt.
                dd_a = junk_pool.tile([P, 1], f32, tag="dd_a")
                act_sq_accum(d_t[:, 0:H], dd_a, slice(0, H))
                dd_v = junk_pool.tile([P, 1], f32, tag="dd_v")
                dve_dot_accum(d_t[:, H:D], d_t[:, H:D], dd_v, slice(H, D))
                dd = junk_pool.tile([P, 1], f32, tag="dd")
                nc.vector.tensor_add(dd[:], dd_a[:], dd_v[:])

                qd = junk_pool.tile([P, 1], f32, tag="qd")
                dve_dot_accum(q_t[:], d_t[:], qd)
                nc.vector.tensor_mul(dots[:, t : t + 1], qd[:], dvb[:, t : t + 1])
                qq = junk_pool.tile([P, 1], f32, tag="qq")
                act_sq_accum(q_t[:], qq)
                nc.vector.tensor_mul(nprod[:, t : t + 1], qq[:], dd[:])

                ad = junk_pool.tile([P, 1], f32, tag="ad")
                dve_dot_accum(a_t[:], d_t[:], ad)
                nc.vector.tensor_mul(
                    dots[:, T + t : T + t + 1], ad[:], dvb[:, T + t : T + t + 1]
                )
                aa = junk_pool.tile([P, 1], f32, tag="aa")
                act_sq_accum(a_t[:], aa)
                nc.vector.tensor_mul(nprod[:, T + t : T + t + 1], aa[:], dd[:])

            # --- Tail: a6 (4 x 1 MB chunks) then a7 (3 x 1.25 MB + one
            # small 256-col chunk so almost no compute trails the final
            # byte).  ad chunks on DVE, aa chunks on ACT, except a7's
            # last aa chunk on DVE so ACT's sqrt-table load starts while
            # the last chunks drain.
            a6 = a_pool.tile([P, D], f32, tag="a")
            a7 = a_pool.tile([P, D], f32, tag="a")
            A7B = [0, 1280, 2560, 3840, D]   # a7 chunk boundaries
            for k in range(NCH):
                sl = slice(k * CH, (k + 1) * CH)
                nc.sync.dma_start(a6[:, sl], samples[bass.ts(T - 2, P), 1, sl])
            for k in range(NCH):
                sl = slice(A7B[k], A7B[k + 1])
                nc.sync.dma_start(a7[:, sl], samples[bass.ts(T - 1, P), 1, sl])

            def chunk_chain(name, emit_one, bounds=None):
                accs = []
                for k in range(NCH):
                    if bounds is None:
                        sl = slice(k * CH, (k + 1) * CH)
                    else:
                        sl = slice(bounds[k], bounds[k + 1])
                    acc = junk_pool.tile([P, 1], f32, tag=f"ch_{name}_{k}")
                    emit_one(k, sl, acc)
                    accs.append(acc)
                    if k > 0:
                        nc.vector.tensor_add(accs[k][:], accs[k][:], accs[k - 1][:])
                return accs[-1]

            ad6 = chunk_chain(
                "ad6", lambda k, sl, acc: dve_dot_accum(a6[:, sl], d6[:, sl], acc, sl)
            )
            aa6 = chunk_chain(
                "aa6", lambda k, sl, acc: act_sq_accum(a6[:, sl], acc, sl)
            )
            ad7 = chunk_chain(
                "ad7",
                lambda k, sl, acc: dve_dot_accum(a7[:, sl], d7[:, sl], acc, sl),
                bounds=A7B,
            )

            def aa7_emit(k, sl, acc):
                if k < NCH - 1:
                    act_sq_accum(a7[:, sl], acc, sl)
                else:
                    dve_dot_accum(a7[:, sl], a7[:, sl], acc, sl)

            aa7 = chunk_chain("aa7", aa7_emit, bounds=A7B)

            c6, c7 = T - 2, T - 1
            nc.vector.tensor_mul(
                dots[:, T + c6 : T + c6 + 1], ad6[:], dvb[:, T + c6 : T + c6 + 1]
            )
            nc.vector.tensor_mul(nprod[:, T + c6 : T + c6 + 1], aa6[:], dd6[:])
            nc.vector.tensor_mul(
                dots[:, T + c7 : T + c7 + 1], ad7[:], dvb[:, T + c7 : T + c7 + 1]
            )
            nc.vector.tensor_mul(nprod[:, T + c7 : T + c7 + 1], aa7[:], dd7[:])

            # --- Batched cos epilogue over all 16 columns (dots already
            # carry the dvb weights): w*cos = dots / max(sqrt(nprod), EPS).
            norm = stats_pool.tile([P, 2 * T], f32, tag="norm")
            nc.scalar.activation(norm[:], nprod[:], Act.Sqrt)
            nc.vector.tensor_scalar_max(norm[:], norm[:], EPS)
            nc.vector.reciprocal(norm[:], norm[:])
            nc.vector.tensor_mul(contrib[:], dots[:], norm[:])

            # Partition reduce: psum[c,0] = sum_p contrib[p,c]; the host
            # sums the 16 column partials per core.
            psum_t = psum_pool.tile([2 * T, 1], f32, tag="psum_s")
            nc.tensor.matmul(psum_t[:], contrib[:], ones[:], start=True, stop=True)
            partial = stats_pool.tile([2 * T, 1], f32, tag="partial")
            nc.vector.tensor_copy(partial[:], psum_t[:])
            nc.sync.dma_start(out[:], partial[:])

    nc.compile()
    return nc


def _get_program():
    if "nc" not in _CACHE:
        _CACHE["nc"] = _build_program()
    return _CACHE["nc"]


def kernel(samples, labels, D_v1, D_v2):
    samples = np.asarray(samples, dtype=np.float32)
    labels = np.asarray(labels, dtype=np.float32)
    D_v1 = np.asarray(D_v1, dtype=np.float32)
    D_v2 = np.asarray(D_v2, dtype=np.float32)
    assert samples.shape == (B, 3, D), samples.shape

    nc = _get_program()

    in_maps = []
    for c in range(N_CORES):
        sl = slice(c * BS, (c + 1) * BS)
        in_maps.append(
            {
                "samples": np.ascontiguousarray(samples[sl]),
                "dv1": np.ascontiguousarray(D_v1[sl]),
                "dv2": np.ascontiguousarray(D_v2[sl]),
            }
        )

    _tc = os.environ.get("KERNEL_TRACE_CORES")
    _kw = {"trace_cores": [int(x) for x in _tc.split(",")]} if _tc else {}
    try:
        res = bass_utils.run_bass_kernel_spmd(
            nc, in_maps, core_ids=list(range(N_CORES)), **_kw
        )
    except Exception:
        # A previously-wedged NeuronCore surfaces as an unrecoverable
        # exec error on the first attempt; the runtime resets it, so a
        # single retry recovers.
        res = bass_utils.run_bass_kernel_spmd(
            nc, in_maps, core_ids=list(range(N_CORES)), **_kw
        )
    _CACHE["last_results"] = res

    # Host-side unshard: sum the per-core column partials into the scalar
    # score, then the scalar BCE.
    score = float(
        sum(
            np.asarray(res.results[c]["out"], dtype=np.float64).sum()
            for c in range(N_CORES)
        )
    )
    y = float(labels.reshape(-1)[0])
    bce = max(score, 0.0) - score * y + np.log1p(np.exp(-abs(score)))
    return np.float32(bce).reshape(())


# revision 13
# speedup vs baseline: 1.0620x; 1.0107x over previous
"""Trainium2 Bass kernel for nn_Discriminator_15668040696127.

Computes:
    q, a, d = samples[:, 0], samples[:, 1], samples[:, 2]        # [B, D]
    cos1 = <q,d> / max(||q||*||d||, 1e-6)                         # [B]
    cos2 = <a,d> / max(||a||*||d||, 1e-6)                         # [B]
    score = cos1 @ D_v1 + cos2 @ D_v2                             # scalar
    out = BCE_with_logits(score, labels[0])                       # scalar

Sharding: data-parallel over B across 8 NeuronCores (1024 samples
each).  Each core streams its 48 MiB sample shard and reduces it to a
single partial-score float; the host sums the 8 partials and applies
the scalar BCE.  No device collective: the SPMD dispatch (one PJRT
shard_map over 8 axon devices) can start cores 100+ us apart, and any
cross-core dependency puts that full skew into every earlier core's
measured exec time (observed 200-315 us run-to-run with an on-device
all-reduce of the same math).

The stream runs anywhere from ~128 us (paired-NC HBM stack idle,
~394 GB/s) to ~148 us, so every engine's per-tile work is kept below
the fast-case per-tile stream time (~15.0 us per 6 MB tile):
  - ACT: qq, aa squares + the low half of dd   (~12.4 us)
  - DVE: qd, ad dots + the high half of dd     (~12.2 us)
(gpsimd only does the tiny dvb loads: TENSOR_SCALAR_PTR is not a
valid Pool opcode, so it cannot take compute passes.)

Tile component order is q,d,a so per-tile DVE work starts as early as
possible and the queue is drained when the tail begins.  d6,q6,d7,q7
are hoisted to the head of the stream (their dd/qq/qd run during
tiles 0..1) and a6/a7 arrive last in chunks (a6: 4 x 1 MB; a7:
3 x 1.25 MB + one 256-col runt), ad->DVE, aa->ACT except a7's last
aa chunk on DVE, so ACT's sqrt-table load and the final chunk passes
overlap and <1.5 us of compute trails the final byte.  dots columns
carry the D_v1/D_v2 weights as they are produced; the cos epilogue is
sqrt -> max -> recip -> mul on [P,16], a [16,1] PE partition-reduce,
and a 64 B output DMA (host sums 8x16 partials + scalar BCE).

Measured (core 0, ntff): 144.8-169.7 us over 4 runs; the spread is
HBM-stack co-tenancy (stream runs 356-397 GB/s run to run).  At equal
stream rate this kernel's fixed overhead is ~17 us (8.1 preamble +
~6.5 tail incl. out-DMA + ~2.5 teardown) vs ~73 us for the baseline.
"""

import os
import sys

import numpy as np

for _p in ("/opt/trn_rl_repo", "/root/.axon_site/_ro/trn_rl_repo"):
    if os.path.isdir(_p) and _p not in sys.path:
        sys.path.append(_p)

import concourse.bass as bass
import concourse.bacc as bacc
import concourse.mybir as mybir
import concourse.tile as tile
from concourse import bass_utils

N_CORES = 8
B, D = 8192, 4096
BS = B // N_CORES          # 1024 samples per core
P = 128                    # SBUF partitions
T = BS // P                # 8 tiles of 128 samples per core
EPS = 1e-6
NCH = 4                    # a-chunks for each of the last two tiles
CH = D // NCH
H = D // 2                 # dd half-split point

f32 = mybir.dt.float32
Alu = mybir.AluOpType
Act = mybir.ActivationFunctionType

_CACHE = {}


def _build_program():
    nc = bacc.Bacc(
        "TRN2",
        target_bir_lowering=False,
        debug=False,
        num_devices=N_CORES,
    )

    samples = nc.dram_tensor("samples", [BS, 3, D], f32, kind="ExternalInput")
    dv1 = nc.dram_tensor("dv1", [BS], f32, kind="ExternalInput")
    dv2 = nc.dram_tensor("dv2", [BS], f32, kind="ExternalInput")
    out = nc.dram_tensor("out", [2 * T, 1], f32, kind="ExternalOutput")

    with tile.TileContext(nc) as tc:
        with (
            tc.tile_pool(name="dp", bufs=3) as d_pool,
            tc.tile_pool(name="qp", bufs=2) as q_pool,
            tc.tile_pool(name="ap", bufs=2) as a_pool,
            tc.tile_pool(name="junk", bufs=1) as junk_pool,
            tc.tile_pool(name="stats", bufs=1) as stats_pool,
            tc.tile_pool(name="psum", bufs=1, space="PSUM") as psum_pool,
        ):
            # Stats columns: col t = tile t's q-vs-d stat, col T+t = a-vs-d.
            dots = stats_pool.tile([P, 2 * T], f32, tag="dots")
            nprod = stats_pool.tile([P, 2 * T], f32, tag="nprod")
            contrib = stats_pool.tile([P, 2 * T], f32, tag="contrib")
            dvb = stats_pool.tile([P, 2 * T], f32, tag="dvb")

            def act_sq_accum(src, acc, sl=None):
                ja = junk_pool.tile([P, D], f32, tag="junk_act")
                o, i = (ja[:], src) if sl is None else (ja[:, sl], src)
                nc.scalar.activation(
                    out=o, in_=i, func=Act.Square, accum_out=acc[:]
                )

            def dve_dot_accum(src0, src1, acc, sl=None):
                jv = junk_pool.tile([P, D], f32, tag="junk_dve")
                o = jv[:] if sl is None else jv[:, sl]
                nc.vector.scalar_tensor_tensor(
                    out=o, in0=src0, scalar=1.0, in1=src1,
                    op0=Alu.mult, op1=Alu.mult, accum_out=acc[:],
                )

            # --- Head: d/q of the last two tiles stream first; their
            # dd/qq/qd work runs while tiles 0..1 stream.
            d6 = stats_pool.tile([P, D], f32, tag="d6")
            q6 = q_pool.tile([P, D], f32, tag="q")
            d7 = stats_pool.tile([P, D], f32, tag="d7")
            q7 = q_pool.tile([P, D], f32, tag="q")
            nc.sync.dma_start(d6[:], samples[bass.ts(T - 2, P), 2, :])
            nc.sync.dma_start(q6[:], samples[bass.ts(T - 2, P), 0, :])
            nc.sync.dma_start(d7[:], samples[bass.ts(T - 1, P), 2, :])
            nc.sync.dma_start(q7[:], samples[bass.ts(T - 1, P), 0, :])

            # Small loads ride the SWDGE path, off the HWDGE stream queue.
            nc.gpsimd.dma_start(dvb[:, 0:T], dv1[:].rearrange("(n p) -> p n", p=P))
            nc.gpsimd.dma_start(
                dvb[:, T : 2 * T], dv2[:].rearrange("(n p) -> p n", p=P)
            )
            ones = stats_pool.tile([P, 1], f32, tag="ones")
            nc.gpsimd.memset(ones[:], 1.0)

            # Head compute: dd6/dd7 whole on ACT (idle in the head),
            # qd6/qd7 on DVE.  dots columns carry the dvb weight already
            # (dot*w) so the tail epilogue skips a [P,16] mul.
            dd6 = stats_pool.tile([P, 1], f32, tag="dd6")
            dd7 = stats_pool.tile([P, 1], f32, tag="dd7")
            for t, qt, dt, ddt in (
                (T - 2, q6, d6, dd6),
                (T - 1, q7, d7, dd7),
            ):
                act_sq_accum(dt[:], ddt)
                qq = junk_pool.tile([P, 1], f32, tag=f"qq{t}")
                act_sq_accum(qt[:], qq)
                qd = junk_pool.tile([P, 1], f32, tag=f"qd{t}")
                dve_dot_accum(qt[:], dt[:], qd)
                nc.vector.tensor_mul(dots[:, t : t + 1], qd[:], dvb[:, t : t + 1])
                nc.vector.tensor_mul(nprod[:, t : t + 1], qq[:], ddt[:])

            # --- Tiles 0..5: q, d, a component DMAs (q first so DVE's qd
            # can start at d-arrival and is long done when a lands).
            for t in range(T - 2):
                q_t = q_pool.tile([P, D], f32, tag="q")
                d_t = d_pool.tile([P, D], f32, tag="d")
                a_t = a_pool.tile([P, D], f32, tag="a")
                nc.sync.dma_start(q_t[:], samples[bass.ts(t, P), 0, :])
                nc.sync.dma_start(d_t[:], samples[bass.ts(t, P), 2, :])
                nc.sync.dma_start(a_t[:], samples[bass.ts(t, P), 1, :])

                # dd split: low half ACT square, high half DVE stt.
                dd_a = junk_pool.tile([P, 1], f32, tag="dd_a")
                act_sq_accum(d_t[:, 0:H], dd_a, slice(0, H))
                dd_v = junk_pool.tile([P, 1], f32, tag="dd_v")
                dve_dot_accum(d_t[:, H:D], d_t[:, H:D], dd_v, slice(H, D))
                dd = junk_pool.tile([P, 1], f32, tag="dd")
                nc.vector.tensor_add(dd[:], dd_a[:], dd_v[:])

                qd = junk_pool.tile([P, 1], f32, tag="qd")
                dve_dot_accum(q_t[:], d_t[:], qd)
                nc.vector.tensor_mul(dots[:, t : t + 1], qd[:], dvb[:, t : t + 1])
                qq = junk_pool.tile([P, 1], f32, tag="qq")
                act_sq_accum(q_t[:], qq)
                nc.vector.tensor_mul(nprod[:, t : t + 1], qq[:], dd[:])

                ad = junk_pool.tile([P, 1], f32, tag="ad")
                dve_dot_accum(a_t[:], d_t[:], ad)
                nc.vector.tensor_mul(
                    dots[:, T + t : T + t + 1], ad[:], dvb[:, T + t : T + t + 1]
                )
                aa = junk_pool.tile([P, 1], f32, tag="aa")
                act_sq_accum(a_t[:], aa)
                nc.vector.tensor_mul(nprod[:, T + t : T + t + 1], aa[:], dd[:])

            # --- Tail: a6 (4 x 1 MB chunks) then a7 (3 x 1.25 MB + one
            # small 256-col chunk so almost no compute trails the final
            # byte).  ad chunks on DVE, aa chunks on ACT, except a7's
            # last aa chunk on DVE so ACT's sqrt-table load starts while
            # the last chunks drain.
            a6 = a_pool.tile([P, D], f32, tag="a")
            a7 = a_pool.tile([P, D], f32, tag="a")
            A7B = [0, 1280, 2560, 3840, D]   # a7 chunk boundaries
            for k in range(NCH):
                sl = slice(k * CH, (k + 1) * CH)
                nc.sync.dma_start(a6[:, sl], samples[bass.ts(T - 2, P), 1, sl])
            for k in range(NCH):
                sl = slice(A7B[k], A7B[k + 1])
                nc.sync.dma_start(a7[:, sl], samples[bass.ts(T - 1, P), 1, sl])

            def chunk_chain(name, emit_one, bounds=None):
                accs = []
                for k in range(NCH):
                    if bounds is None:
                        sl = slice(k * CH, (k + 1) * CH)
                    else:
                        sl = slice(bounds[k], bounds[k + 1])
                    acc = junk_pool.tile([P, 1], f32, tag=f"ch_{name}_{k}")
                    emit_one(k, sl, acc)
                    accs.append(acc)
                    if k > 0:
                        nc.vector.tensor_add(accs[k][:], accs[k][:], accs[k - 1][:])
                return accs[-1]

            c6, c7 = T - 2, T - 1
            ad6 = chunk_chain(
                "ad6", lambda k, sl, acc: dve_dot_accum(a6[:, sl], d6[:, sl], acc, sl)
            )
            aa6 = chunk_chain(
                "aa6", lambda k, sl, acc: act_sq_accum(a6[:, sl], acc, sl)
            )
            nc.vector.tensor_mul(
                dots[:, T + c6 : T + c6 + 1], ad6[:], dvb[:, T + c6 : T + c6 + 1]
            )
            nc.vector.tensor_mul(nprod[:, T + c6 : T + c6 + 1], aa6[:], dd6[:])

            # a7: ad chunks 0..2 on DVE at arrival; aa chunks 0..2 on ACT
            # with the runt aa chunk on DVE feeding nprod col 15 (the SQRT
            # gate) FIRST, so the runt ad chunk + weighted-dots mul (needed
            # only after the reciprocal) overlap the ACT sqrt.
            ad7_accs = []
            for k in range(NCH - 1):
                sl = slice(A7B[k], A7B[k + 1])
                acc = junk_pool.tile([P, 1], f32, tag=f"ch_ad7_{k}")
                dve_dot_accum(a7[:, sl], d7[:, sl], acc, sl)
                if k > 0:
                    nc.vector.tensor_add(acc[:], acc[:], ad7_accs[-1][:])
                ad7_accs.append(acc)

            def aa7_emit(k, sl, acc):
                if k < NCH - 1:
                    act_sq_accum(a7[:, sl], acc, sl)
                else:
                    dve_dot_accum(a7[:, sl], a7[:, sl], acc, sl)

            aa7 = chunk_chain("aa7", aa7_emit, bounds=A7B)
            nc.vector.tensor_mul(nprod[:, T + c7 : T + c7 + 1], aa7[:], dd7[:])

            sl = slice(A7B[NCH - 1], A7B[NCH])
            ad7 = junk_pool.tile([P, 1], f32, tag=f"ch_ad7_{NCH - 1}")
            dve_dot_accum(a7[:, sl], d7[:, sl], ad7, sl)
            nc.vector.tensor_add(ad7[:], ad7[:], ad7_accs[-1][:])
            nc.vector.tensor_mul(
                dots[:, T + c7 : T + c7 + 1], ad7[:], dvb[:, T + c7 : T + c7 + 1]
            )

            # --- Batched cos epilogue over all 16 columns (dots already
            # carry the dvb weights): w*cos = dots / max(sqrt(nprod), EPS).
            norm = stats_pool.tile([P, 2 * T], f32, tag="norm")
            nc.scalar.activation(norm[:], nprod[:], Act.Sqrt)
            nc.vector.tensor_scalar_max(norm[:], norm[:], EPS)
            nc.vector.reciprocal(norm[:], norm[:])
            nc.vector.tensor_mul(contrib[:], dots[:], norm[:])

            # Partition reduce: psum[c,0] = sum_p contrib[p,c]; the host
            # sums the 16 column partials per core.
            psum_t = psum_pool.tile([2 * T, 1], f32, tag="psum_s")
            nc.tensor.matmul(psum_t[:], contrib[:], ones[:], start=True, stop=True)
            partial = stats_pool.tile([2 * T, 1], f32, tag="partial")
            nc.vector.tensor_copy(partial[:], psum_t[:])
            nc.sync.dma_start(out[:], partial[:])

    nc.compile()
    return nc


def _get_program():
    if "nc" not in _CACHE:
        _CACHE["nc"] = _build_program()
    return _CACHE["nc"]


def kernel(samples, labels, D_v1, D_v2):
    samples = np.asarray(samples, dtype=np.float32)
    labels = np.asarray(labels, dtype=np.float32)
    D_v1 = np.asarray(D_v1, dtype=np.float32)
    D_v2 = np.asarray(D_v2, dtype=np.float32)
    assert samples.shape == (B, 3, D), samples.shape

    nc = _get_program()

    in_maps = []
    for c in range(N_CORES):
        sl = slice(c * BS, (c + 1) * BS)
        in_maps.append(
            {
                "samples": np.ascontiguousarray(samples[sl]),
                "dv1": np.ascontiguousarray(D_v1[sl]),
                "dv2": np.ascontiguousarray(D_v2[sl]),
            }
        )

    _tc = os.environ.get("KERNEL_TRACE_CORES")
    _kw = {"trace_cores": [int(x) for x in _tc.split(",")]} if _tc else {}
    try:
        res = bass_utils.run_bass_kernel_spmd(
            nc, in_maps, core_ids=list(range(N_CORES)), **_kw
        )
    except Exception:
        # A previously-wedged NeuronCore surfaces as an unrecoverable
        # exec error on the first attempt; the runtime resets it, so a
        # single retry recovers.
        res = bass_utils.run_bass_kernel_spmd(
            nc, in_maps, core_ids=list(range(N_CORES)), **_kw
        )
    _CACHE["last_results"] = res

    # Host-side unshard: sum the per-core column partials into the scalar
    # score, then the scalar BCE.
    score = float(
        sum(
            np.asarray(res.results[c]["out"], dtype=np.float64).sum()
            for c in range(N_CORES)
        )
    )
    y = float(labels.reshape(-1)[0])
    bce = max(score, 0.0) - score * y + np.log1p(np.exp(-abs(score)))
    return np.float32(bce).reshape(())
